# revision 45
# baseline (speedup 1.0000x reference)
"""Multi-head attention (B=4, S=2048, D=1024, H=16, dk=dv=64) on 8 Trainium2
NeuronCores.

Sharding: core c handles batch b = c//2 and head-group g = c%2 (8 of 16 heads).
Per core (bf16 data path, fp32 PSUM accumulation):
  - Inputs X and weights W arrive in bf16 (host-converted): DMA loads them
    directly, PE-transposes the (2048, 1024) inputs in bf16.
  - K and Q project to K^T/Q^T [512, 2048] (head-dim on partitions), biases
    folded into the PSUM->SBUF copies (bf16 out). V projects DIRECTLY to
    natural [2048, 8x(64+1)] per 128-row key tile (X^T block stationary, Wv
    moving), bias added via a host-replicated [128, 512] tile on the
    PSUM->SBUF copy; a ones column per head makes the PV matmul also produce
    softmax row-sums.
  - Per head-pair, 512-wide query chunk, key tile: scores^T = K Q^T via
    row-tiled K=64 bf16 matmuls (array rows 0:64 / 64:128) into fp32 PSUM,
    exp on the scalar engine straight out of PSUM (mask as the per-partition
    bias, 1/sqrt(dk) as the scale) writing bf16, PV accumulated in fp32
    PSUM over the 16 key tiles.
  - Context (+rowsum row) is PE-transposed (bf16) back to natural,
    normalized with the reciprocal rowsums, DMA'd out as bf16 `weights`
    (host casts to f32), then transposed once more to head-dim-major (bf16)
    as the stationary operand of the output projection. o_proj bias is
    added on the host during the cross-core reduction.
  - Pipeline balance: phase B runs the scalar engine (softmax exp, ~1.09us
    per [128,1024] tile) and the PE (scores+PV+deferred chunks) at
    equilibrium (~1.35us per key-tile iteration each). Post-processing
    (normalize/re-transpose/o_proj) and the Q blk-1 projection are emitted
    as ~1-2us deferred chunks popped every 4th key tile, which is exactly
    the PE slack left over by the exp stream -- deferring more work into
    phase B starves the scalar engine and regresses (measured).
Host: slices inputs per core (casting X, W to bf16), sums the o_proj
partials of each core pair plus bo, and concatenates the weights halves.

Measured end-to-end relative error ~9.7e-3 (tolerance 2e-2), deterministic
for the fixed-seed harness inputs.
"""
import sys

for _p in ("/opt/trn_rl_repo", "/root/.axon_site/_ro/trn_rl_repo"):
    if _p not in sys.path:
        sys.path.insert(0, _p)

import numpy as np
import ml_dtypes
import concourse.bass as bass
import concourse.bacc as bacc
import concourse.tile as tile
from concourse import mybir
from concourse.masks import make_identity
from concourse.bass_utils import run_bass_kernel_spmd

F32 = mybir.dt.float32
BF16 = mybir.dt.bfloat16
NP_BF16 = ml_dtypes.bfloat16
EXP = mybir.ActivationFunctionType.Exp
ADD = mybir.AluOpType.add
MULT = mybir.AluOpType.mult
BYPASS = mybir.AluOpType.bypass

B, S, D = 4, 2048, 1024
H, DK, DV = 16, 64, 64
NCORES = 8
HC = H // 2          # heads per core
HDK = HC * DK        # 512 head dims per core
SQC = 512            # query-chunk width


def build_program(nc: bass.Bass, s=S, d=D, hc=HC):
    hdk = hc * DK
    ck_n = hdk // 128        # dk partition-tiles (= head pairs)
    dt_n = d // 128          # D contraction tiles
    skt_n = s // 128         # key tiles
    sq_n = s // SQC          # query chunks
    ab = min(2 * SQC, s)     # phase-A S-block width
    abn = s // ab

    xq = nc.dram_tensor("xq", [s, d], BF16, kind="ExternalInput")
    xk = nc.dram_tensor("xk", [s, d], BF16, kind="ExternalInput")
    xv = nc.dram_tensor("xv", [s, d], BF16, kind="ExternalInput")
    wq = nc.dram_tensor("wq", [d, hdk], BF16, kind="ExternalInput")
    wk = nc.dram_tensor("wk", [d, hdk], BF16, kind="ExternalInput")
    wv = nc.dram_tensor("wv", [d, hdk], BF16, kind="ExternalInput")
    bq = nc.dram_tensor("bq", [ck_n, 128, 1], F32, kind="ExternalInput")
    bk = nc.dram_tensor("bk", [ck_n, 128, 1], F32, kind="ExternalInput")
    bvf = nc.dram_tensor("bvf", [128, hdk], BF16, kind="ExternalInput")
    wo = nc.dram_tensor("wo", [hdk, d], BF16, kind="ExternalInput")
    msk = nc.dram_tensor("msk", [skt_n, 128, 1], F32, kind="ExternalInput")

    out_p = nc.dram_tensor("out_p", [s, d], F32, kind="ExternalOutput")
    wts_p = nc.dram_tensor("wts_p", [s, hdk], BF16, kind="ExternalOutput")

    # weights DMA view: rows (q, z, p), cols (pair j, head m, dv)
    wts_v = wts_p.rearrange(
        "(q z p) (j m e) -> q j p m z e", z=SQC // 128, p=128, m=2, e=DV
    )

    with tile.TileContext(nc) as tc, \
            tc.tile_pool(name="consts", bufs=1) as consts, \
            tc.tile_pool(name="persist", bufs=1) as persist:
        identb = consts.tile([128, 128], BF16, name="identb")
        make_identity(nc, identb)
        msk_sb = consts.tile([128, skt_n], F32, name="msk_sb")
        nc.sync.dma_start(out=msk_sb, in_=msk.rearrange("t p one -> p (t one)"))
        bias_t = {}
        for nm, src in (("q", bq), ("k", bk)):
            bt = consts.tile([128, ck_n], F32, name=f"b{nm}_t")
            nc.sync.dma_start(out=bt, in_=src.rearrange("t p one -> p (t one)"))
            bias_t[nm] = bt
        bvf_sb = consts.tile([128, hdk], BF16, name="bvf_sb")
        nc.sync.dma_start(out=bvf_sb, in_=bvf[:])

        qT = persist.tile([128, ck_n, s], BF16, name="qT")
        kT = persist.tile([128, ck_n, s], BF16, name="kT")
        vtn = persist.tile([128, skt_n, hc, DV + 1], BF16, name="vtn")
        ones_th = consts.tile([128, skt_n * hc], BF16, name="ones_th")
        nc.gpsimd.memset(ones_th, 1.0)
        nc.vector.tensor_copy(
            vtn[:, :, :, DV : DV + 1],
            ones_th.rearrange("p (t h one) -> p t h one", t=skt_n, one=1),
        )
        wo_sb = persist.tile([128, ck_n, d], BF16, name="wo_sb")
        # QKV weights + blk-1 transposed-X staging persist into phase B: the
        # blk-1 projections are deferred into the attention pending queue so
        # the softmax stream starts earlier.
        w_sbs = {
            nm: persist.tile([128, dt_n, hdk], BF16, name=f"w{nm}_sb")
            for nm in ("q", "k", "v")
        }
        xTq = persist.tile([128, dt_n, s], BF16, name="xTq")
        xTk = persist.tile([128, dt_n, s], BF16, name="xTk")

        # ---------------- Phase A: transposes + projections -----------------
        # K first, then V, then Q: attention on early query chunks only
        # needs kT + vtn + the first Q block.
        with (
            tc.tile_pool(name="xTv", bufs=1) as xTv_pool,
            tc.tile_pool(name="pa_ps", bufs=2, space="PSUM") as pa_ps,
        ):
            def proj_block(nm, outT, ck, src, c0):
                # project 1024 columns [c0, c0+ab) of src into outT[:, ck]
                pp = pa_ps.tile([128, ab], F32, name="proj_pp")
                for dt_ in range(dt_n):
                    lhsT = w_sbs[nm][:, dt_, ck * 128 : (ck + 1) * 128]
                    for h2 in range(ab // 512):
                        nc.tensor.matmul(
                            pp[:, h2 * 512 : (h2 + 1) * 512],
                            lhsT,
                            src[:, dt_, c0 + h2 * 512 : c0 + (h2 + 1) * 512],
                            start=(dt_ == 0),
                            stop=(dt_ == dt_n - 1),
                        )
                nc.vector.tensor_scalar(
                    out=outT[:, ck, c0 : c0 + ab],
                    in0=pp,
                    scalar1=bias_t[nm][:, ck : ck + 1],
                    scalar2=None,
                    op0=ADD,
                )

            # X^T via DGE crossbar transpose DMAs (bf16) -- no PE transposes,
            # no PSUM staging, no copy traffic. Weight DMAs interleave so
            # each input's weights land just before its projection starts.
            xTv = xTv_pool.tile([128, dt_n, s], BF16, name="xTv")
            for nm, wz, xz, dst in (
                ("k", wk, xk, xTk), ("v", wv, xv, xTv), ("q", wq, xq, xTq)
            ):
                for t in range(dt_n):
                    nc.sync.dma_start(
                        out=w_sbs[nm][:, t], in_=wz[t * 128 : (t + 1) * 128, :]
                    )
                for dt_ in range(dt_n):
                    nc.sync.dma_start(
                        out=dst[:, dt_],
                        in_=xz[:, dt_ * 128 : (dt_ + 1) * 128],
                        transpose=True,
                    )
            for t in range(ck_n):
                nc.sync.dma_start(out=wo_sb[:, t], in_=wo[t * 128 : (t + 1) * 128, :])
            # K: fully projected here
            for blk in range(abn):
                for ck in range(ck_n):
                    proj_block("k", kT, ck, xTk, blk * ab)
            # V: fully projected here (every attention iteration reads all
            # key tiles, so V cannot be deferred without starving softmax).
            for st in range(s // 128):
                ppv = pa_ps.tile([128, ab], F32, name="proj_pp")[:, 0:hdk]
                for dt_ in range(dt_n):
                    nc.tensor.matmul(
                        ppv,
                        xTv[:, dt_, st * 128 : (st + 1) * 128],
                        w_sbs["v"][:, dt_],
                        start=(dt_ == 0),
                        stop=(dt_ == dt_n - 1),
                    )
                nc.vector.scalar_tensor_tensor(
                    out=vtn[:, st, :, 0:DV],
                    in0=ppv.rearrange("p (h e) -> p h e", h=hc),
                    scalar=0.0,
                    in1=bvf_sb.rearrange("p (h e) -> p h e", h=hc),
                    op0=BYPASS,
                    op1=ADD,
                )
            # Q: blk 0 projected here, blk 1 deferred into the pending queue
            for ck in range(ck_n):
                proj_block("q", qT, ck, xTq, 0)

        # ---------------- Phase B: attention + o_proj -----------------------
        zn = SQC // 128
        with (
            tc.tile_pool(name="ep", bufs=8) as ep_pool,
            tc.tile_pool(name="ctxu", bufs=3) as ctxu_pool,
            tc.tile_pool(name="wtsT", bufs=2) as wtsT_pool,
            tc.tile_pool(name="wnat", bufs=3) as wnat_pool,
            tc.tile_pool(name="rcp", bufs=3) as rcp_pool,
            tc.tile_pool(name="outsb", bufs=3) as outsb_pool,
            tc.tile_pool(name="sc_ps", bufs=2, space="PSUM") as sc_ps,
            tc.tile_pool(name="ctx_ps", bufs=2, space="PSUM") as ctx_ps,
            tc.tile_pool(name="aux_ps", bufs=2, space="PSUM") as aux_ps,
        ):
            # Deferred post-processing (normalization / re-transposes /
            # o_proj), emitted in ~1-2us chunks interleaved into later
            # iterations' attention loops. They draw PSUM from a dedicated
            # 2-slot aux pool so they overlap the scores/exp/PV pipeline
            # instead of stalling it.
            pending = []

            def proj_chunk(nm, outT, ck, src, c0):
                # deferred 512-column projection chunk: outT[:, ck, c0:c0+512]
                def emit():
                    pp = aux_ps.tile([128, 512], F32, name="aux")
                    for dt_ in range(dt_n):
                        nc.tensor.matmul(
                            pp,
                            w_sbs[nm][:, dt_, ck * 128 : (ck + 1) * 128],
                            src[:, dt_, c0 : c0 + 512],
                            start=(dt_ == 0), stop=(dt_ == dt_n - 1),
                        )
                    nc.vector.tensor_scalar(
                        out=outT[:, ck, c0 : c0 + 512],
                        in0=pp,
                        scalar1=bias_t[nm][:, ck : ck + 1],
                        scalar2=None,
                        op0=ADD,
                    )
                return emit

            # Deferred Q blk-1 projections (first needed by q-chunk 2;
            # drained via the t%4 pop cadence well before that)
            for ck in range(ck_n):
                pending.append(proj_chunk("q", qT, ck, xTq, 1024))
                pending.append(proj_chunk("q", qT, ck, xTq, 1536))

            def weights_chunk(q, j, m, ctxu, wnat, rc, wtsT_sb):
                def emit():
                    # padded to DV+2 so each zz slice starts 4-byte aligned
                    nat = aux_ps.tile([128, zn, DV + 2], BF16, name="aux")
                    for zz in range(zn):
                        nc.tensor.transpose(
                            nat[:, zz, 0 : DV + 1],
                            ctxu[:, m * SQC + zz * 128 : m * SQC + (zz + 1) * 128],
                            identb[0 : DV + 1, 0 : DV + 1],
                        )
                    nc.vector.reciprocal(rc[:, m], nat[:, :, DV : DV + 1])
                    for zz in range(zn):
                        nc.vector.tensor_scalar(
                            out=wnat[:, m, zz],
                            in0=nat[:, zz, 0:DV],
                            scalar1=rc[:, m, zz],
                            scalar2=None,
                            op0=MULT,
                        )
                    # normalized natural -> head-dim-major (o_proj lhsT).
                    # Both heads: transpose-mode landing at psum partition 0;
                    # head B's copy shifts partitions 0:64 -> 64:128.
                    wtp = aux_ps.tile([64, zn, 128], BF16, name="aux")
                    for zz in range(zn):
                        nc.tensor.transpose(wtp[:, zz], wnat[:, m, zz], identb)
                    nc.vector.tensor_copy(
                        wtsT_sb[m * 64 : m * 64 + 64, j, :], wtp
                    )
                    nc.sync.dma_start(out=wts_v[q, j, :, m], in_=wnat[:, m])
                return emit

            def oproj_chunk(q, zz, h2, wtsT_sb, out_sb):
                def emit():
                    op = aux_ps.tile([128, 512], F32, name="aux")
                    for dt_ in range(ck_n):
                        nc.tensor.matmul(
                            op,
                            wtsT_sb[:, dt_, zz * 128 : (zz + 1) * 128],
                            wo_sb[:, dt_, h2 * 512 : (h2 + 1) * 512],
                            start=(dt_ == 0), stop=(dt_ == ck_n - 1),
                        )
                    nc.vector.tensor_copy(out_sb[:, h2 * 512 : (h2 + 1) * 512], op)
                    if h2 == d // 512 - 1:
                        r0 = q * SQC + zz * 128
                        nc.sync.dma_start(out=out_p[r0 : r0 + 128, :], in_=out_sb)
                return emit

            for q in range(sq_n):
                q0 = q * SQC
                wtsT_sb = wtsT_pool.tile([128, ck_n, SQC], BF16, name="wtsT_sb")
                for j in range(ck_n):
                    ctxA = ctx_ps.tile([DV + 1, SQC], F32, name="ctx_t")
                    ctxB = ctx_ps.tile([DV + 1, SQC], F32, name="ctx_t")
                    for t in range(skt_n):
                        sc = sc_ps.tile([128, 2 * SQC], F32, name="sc_t")
                        for m in range(2):
                            lo, hi = m * 64, (m + 1) * 64
                            nc.tensor.matmul(
                                sc[:, m * SQC : (m + 1) * SQC],
                                kT[lo:hi, j, t * 128 : (t + 1) * 128],
                                qT[lo:hi, j, q0 : q0 + SQC],
                                start=True, stop=True,
                                tile_position=(m * 64, 0),
                            )
                        ep = ep_pool.tile([128, 2 * SQC], BF16, name="ep_t")
                        nc.scalar.activation(
                            ep, sc, EXP, bias=msk_sb[:, t : t + 1], scale=0.125
                        )
                        nc.tensor.matmul(
                            ctxA, vtn[:, t, 2 * j], ep[:, 0:SQC],
                            start=(t == 0), stop=(t == skt_n - 1),
                        )
                        nc.tensor.matmul(
                            ctxB, vtn[:, t, 2 * j + 1], ep[:, SQC : 2 * SQC],
                            start=(t == 0), stop=(t == skt_n - 1),
                        )
                        if t % 4 == 3 and pending:
                            pending.pop(0)()

                    ctxu = ctxu_pool.tile([DV + 1, 2 * SQC], BF16, name="ctxu_t")
                    nc.vector.tensor_copy(ctxu[:, 0:SQC], ctxA)
                    nc.vector.tensor_copy(ctxu[:, SQC : 2 * SQC], ctxB)
                    wnat = wnat_pool.tile([128, 2, zn, DV], BF16, name="wnat_t")
                    rc = rcp_pool.tile([128, 2, zn, 1], F32, name="rc_t")
                    for m in range(2):
                        pending.append(
                            weights_chunk(q, j, m, ctxu, wnat, rc, wtsT_sb)
                        )
                for zz in range(zn):
                    out_sb = outsb_pool.tile([128, d], F32, name="out_sb")
                    for h2 in range(d // 512):
                        pending.append(oproj_chunk(q, zz, h2, wtsT_sb, out_sb))
            while pending:
                pending.pop(0)()
    return nc


_CACHE = {}


def _get_program():
    if "nc" not in _CACHE:
        nc = bacc.Bacc("TRN2")
        build_program(nc)
        nc.compile()
        _CACHE["nc"] = nc
    return _CACHE["nc"]


def kernel(query, key, value, mask, Wq, bq, Wk, bk, Wv, bv, Wo, bo, trace=False):
    f32 = lambda a: np.ascontiguousarray(a, dtype=np.float32)
    bf16 = lambda a: np.ascontiguousarray(np.asarray(a, dtype=np.float32), dtype=NP_BF16)
    query, key, value, mask = bf16(query), bf16(key), bf16(value), f32(mask)
    Wq, Wk, Wv, Wo = bf16(Wq), bf16(Wk), bf16(Wv), bf16(Wo)
    bq, bk, bv, bo = f32(bq), f32(bk), f32(bv), f32(bo)

    in_maps = []
    for c in range(NCORES):
        b, g = c // 2, c % 2
        cols = slice(g * HDK, (g + 1) * HDK)
        bvf = np.ascontiguousarray(
            np.broadcast_to(bv[cols].astype(NP_BF16), (128, HDK))
        )
        in_maps.append({
            "xq": query[b], "xk": key[b], "xv": value[b],
            "wq": np.ascontiguousarray(Wq[:, cols]),
            "wk": np.ascontiguousarray(Wk[:, cols]),
            "wv": np.ascontiguousarray(Wv[:, cols]),
            "bq": bq[cols].reshape(HDK // 128, 128, 1),
            "bk": bk[cols].reshape(HDK // 128, 128, 1),
            "bvf": bvf,
            "wo": np.ascontiguousarray(Wo[cols, :]),
            "msk": mask[b, 0, 0].reshape(S // 128, 128, 1),
        })

    nc = _get_program()
    res = run_bass_kernel_spmd(
        nc, in_maps, core_ids=list(range(NCORES)), trace=trace
    )

    output = np.empty((B, S, D), np.float32)
    weights = np.empty((B, S, H * DV), np.float32)
    for b in range(B):
        output[b] = res.results[2 * b]["out_p"] + res.results[2 * b + 1]["out_p"] + bo
        weights[b, :, 0:HDK] = res.results[2 * b]["wts_p"].astype(np.float32)
        weights[b, :, HDK:] = res.results[2 * b + 1]["wts_p"].astype(np.float32)
    if trace:
        _CACHE["last_exec_time_ns"] = res.exec_time_ns
        _CACHE["last_res"] = res
    return output, weights


# revision 50
# speedup vs baseline: 1.1106x; 1.1106x over previous
"""Multi-head attention (B=4, S=2048, D=1024, H=16, dk=dv=64) on 8 Trainium2
NeuronCores.

Sharding: core c handles batch b = c//2 and head-group g = c%2 (8 of 16 heads).
Per core (bf16 data path, fp32 PSUM accumulation):
  - Inputs X and weights W arrive in bf16 (host-converted): DMA loads them
    directly, PE-transposes the (2048, 1024) inputs in bf16.
  - K and Q project to K^T/Q^T [512, 2048] (head-dim on partitions), biases
    folded into the PSUM->SBUF copies (bf16 out). V projects DIRECTLY to
    natural [2048, 8x(64+1)] per 128-row key tile (X^T block stationary, Wv
    moving), bias added via a host-replicated [128, 512] tile on the
    PSUM->SBUF copy; a ones column per head makes the PV matmul also produce
    softmax row-sums.
  - Per head-pair, 512-wide query chunk, key tile: scores^T = K Q^T via
    row-tiled K=64 bf16 matmuls (array rows 0:64 / 64:128) into fp32 PSUM,
    exp on the scalar engine straight out of PSUM (mask as the per-partition
    bias, 1/sqrt(dk) as the scale) writing bf16, PV accumulated in fp32
    PSUM over the 16 key tiles.
  - Context (+rowsum row) is PE-transposed (bf16) back to natural,
    normalized with the reciprocal rowsums, DMA'd out as bf16 `weights`
    (host casts to f32), then transposed once more to head-dim-major (bf16)
    as the stationary operand of the output projection. o_proj bias is
    added on the host during the cross-core reduction.
  - Pipeline balance: phase B runs the scalar engine (softmax exp, ~1.09us
    per [128,1024] tile) and the PE (scores+PV+deferred chunks) at
    equilibrium (~1.35us per key-tile iteration each). Post-processing
    (normalize/re-transpose/o_proj) and the Q blk-1 projection are emitted
    as ~1-2us deferred chunks popped every 4th key tile, which is exactly
    the PE slack left over by the exp stream -- deferring more work into
    phase B starves the scalar engine and regresses (measured).
Host: slices inputs per core (casting X, W to bf16), sums the o_proj
partials of each core pair plus bo, and concatenates the weights halves.

Measured end-to-end relative error ~9.7e-3 (tolerance 2e-2), deterministic
for the fixed-seed harness inputs.
"""
import sys

for _p in ("/opt/trn_rl_repo", "/root/.axon_site/_ro/trn_rl_repo"):
    if _p not in sys.path:
        sys.path.insert(0, _p)

import numpy as np
import ml_dtypes
import concourse.bass as bass
import concourse.bacc as bacc
import concourse.tile as tile
from concourse import mybir
from concourse.masks import make_identity
from concourse.bass_utils import run_bass_kernel_spmd

F32 = mybir.dt.float32
BF16 = mybir.dt.bfloat16
NP_BF16 = ml_dtypes.bfloat16
EXP = mybir.ActivationFunctionType.Exp
ADD = mybir.AluOpType.add
MULT = mybir.AluOpType.mult
BYPASS = mybir.AluOpType.bypass

B, S, D = 4, 2048, 1024
H, DK, DV = 16, 64, 64
NCORES = 8
HC = H // 2          # heads per core
HDK = HC * DK        # 512 head dims per core
SQC = 512            # query-chunk width


def build_program(nc: bass.Bass, s=S, d=D, hc=HC):
    hdk = hc * DK
    ck_n = hdk // 128        # dk partition-tiles (= head pairs)
    dt_n = d // 128          # D contraction tiles
    skt_n = s // 128         # key tiles
    sq_n = s // SQC          # query chunks
    ab = min(2 * SQC, s)     # phase-A S-block width
    abn = s // ab

    xq = nc.dram_tensor("xq", [s, d], BF16, kind="ExternalInput")
    xk = nc.dram_tensor("xk", [s, d], BF16, kind="ExternalInput")
    xv = nc.dram_tensor("xv", [s, d], BF16, kind="ExternalInput")
    wq = nc.dram_tensor("wq", [d, hdk], BF16, kind="ExternalInput")
    wk = nc.dram_tensor("wk", [d, hdk], BF16, kind="ExternalInput")
    wv = nc.dram_tensor("wv", [d, hdk], BF16, kind="ExternalInput")
    bq = nc.dram_tensor("bq", [ck_n, 128, 1], F32, kind="ExternalInput")
    bk = nc.dram_tensor("bk", [ck_n, 128, 1], F32, kind="ExternalInput")
    bvf = nc.dram_tensor("bvf", [128, hdk], BF16, kind="ExternalInput")
    wo = nc.dram_tensor("wo", [hdk, d], BF16, kind="ExternalInput")
    msk = nc.dram_tensor("msk", [skt_n, 128, 1], F32, kind="ExternalInput")

    out_p = nc.dram_tensor("out_p", [s, d], F32, kind="ExternalOutput")
    wts_p = nc.dram_tensor("wts_p", [s, hdk], BF16, kind="ExternalOutput")

    # weights DMA view: rows (q, z, p), cols (pair j, head m, dv)
    wts_v = wts_p.rearrange(
        "(q z p) (j m e) -> q j p m z e", z=SQC // 128, p=128, m=2, e=DV
    )

    with tile.TileContext(nc) as tc, \
            tc.tile_pool(name="consts", bufs=1) as consts, \
            tc.tile_pool(name="persist", bufs=1) as persist:
        identb = consts.tile([128, 128], BF16, name="identb")
        make_identity(nc, identb)
        msk_sb = consts.tile([128, skt_n], F32, name="msk_sb")
        nc.sync.dma_start(out=msk_sb, in_=msk.rearrange("t p one -> p (t one)"))
        bias_t = {}
        for nm, src in (("q", bq), ("k", bk)):
            bt = consts.tile([128, ck_n], F32, name=f"b{nm}_t")
            nc.sync.dma_start(out=bt, in_=src.rearrange("t p one -> p (t one)"))
            bias_t[nm] = bt
        bvf_sb = consts.tile([128, hdk], BF16, name="bvf_sb")
        nc.sync.dma_start(out=bvf_sb, in_=bvf[:])

        qT = persist.tile([128, ck_n, s], BF16, name="qT")
        kT = persist.tile([128, ck_n, s], BF16, name="kT")
        vtn = persist.tile([128, skt_n, hc, DV + 1], BF16, name="vtn")
        ones_th = consts.tile([128, skt_n * hc], BF16, name="ones_th")
        nc.gpsimd.memset(ones_th, 1.0)
        nc.vector.tensor_copy(
            vtn[:, :, :, DV : DV + 1],
            ones_th.rearrange("p (t h one) -> p t h one", t=skt_n, one=1),
        )
        wo_sb = persist.tile([128, ck_n, d], BF16, name="wo_sb")
        for t in range(ck_n):
            nc.sync.dma_start(out=wo_sb[:, t], in_=wo[t * 128 : (t + 1) * 128, :])
        # QKV weights + blk-1 transposed-X staging persist into phase B: the
        # blk-1 projections are deferred into the attention pending queue so
        # the softmax stream starts earlier.
        w_sbs = {}
        for nm, wz in (("k", wk), ("v", wv), ("q", wq)):
            wt = persist.tile([128, dt_n, hdk], BF16, name=f"w{nm}_sb")
            for t in range(dt_n):
                nc.sync.dma_start(out=wt[:, t], in_=wz[t * 128 : (t + 1) * 128, :])
            w_sbs[nm] = wt
        xTq = persist.tile([128, dt_n, s], BF16, name="xTq")
        xTk = persist.tile([128, dt_n, s], BF16, name="xTk")

        # ---------------- Phase A: transposes + projections -----------------
        # K first, then V, then Q: attention on early query chunks only
        # needs kT + vtn + the first Q block.
        ncopy = 0
        with (
            tc.tile_pool(name="xnat", bufs=5) as xnat_pool,
            tc.tile_pool(name="xT", bufs=2) as xT_pool,
            tc.tile_pool(name="pa_ps", bufs=2, space="PSUM") as pa_ps,
            tc.tile_pool(name="tr_ps", bufs=3, space="PSUM") as tr_ps,
        ):
            def do_transposes(xz, blk, dst_sb, dst_c0):
                # transpose rows [blk*ab, (blk+1)*ab) of xz into dst_sb
                # (layout [128, dt_n, cols]) starting at column dst_c0
                nonlocal ncopy
                for st in range(ab // 128):
                    x_sub = xnat_pool.tile([128, d], BF16, name="x_sub")
                    s0 = blk * ab + st * 128
                    nc.sync.dma_start(out=x_sub, in_=xz[s0 : s0 + 128, :])
                    for tg in range(dt_n // 4):
                        tp = tr_ps.tile([128, 4, 128], BF16, name="tr_tp")
                        for j in range(4):
                            nc.tensor.transpose(
                                tp[:, j],
                                x_sub[:, (tg * 4 + j) * 128 : (tg * 4 + j + 1) * 128],
                                identb,
                            )
                        c0 = dst_c0 + st * 128
                        dst = dst_sb[:, tg * 4 : tg * 4 + 4, c0 : c0 + 128]
                        if ncopy % 2 == 0:
                            nc.vector.tensor_copy(dst, tp)
                        else:
                            nc.scalar.copy(dst, tp)
                        ncopy += 1

            def proj_block(nm, outT, ck, src, c0):
                # project 1024 columns [c0, c0+ab) of src into outT[:, ck]
                pp = pa_ps.tile([128, ab], F32, name="proj_pp")
                for dt_ in range(dt_n):
                    lhsT = w_sbs[nm][:, dt_, ck * 128 : (ck + 1) * 128]
                    for h2 in range(ab // 512):
                        nc.tensor.matmul(
                            pp[:, h2 * 512 : (h2 + 1) * 512],
                            lhsT,
                            src[:, dt_, c0 + h2 * 512 : c0 + (h2 + 1) * 512],
                            start=(dt_ == 0),
                            stop=(dt_ == dt_n - 1),
                        )
                nc.vector.tensor_scalar(
                    out=outT[:, ck, c0 : c0 + ab],
                    in0=pp,
                    scalar1=bias_t[nm][:, ck : ck + 1],
                    scalar2=None,
                    op0=ADD,
                )

            # K: fully projected here
            do_transposes(xk, 0, xTk, 0)
            do_transposes(xk, 1, xTk, ab)
            for blk in range(abn):
                for ck in range(ck_n):
                    proj_block("k", kT, ck, xTk, blk * ab)
            # V: fully projected here (every attention iteration reads all
            # key tiles, so V cannot be deferred without starving softmax).
            for blk in range(abn):
                xTv = xT_pool.tile([128, dt_n, ab], BF16, name="xT_sb")
                do_transposes(xv, blk, xTv, 0)
                for st in range(ab // 128):
                    ppv = pa_ps.tile([128, ab], F32, name="proj_pp")[:, 0:hdk]
                    for dt_ in range(dt_n):
                        nc.tensor.matmul(
                            ppv,
                            xTv[:, dt_, st * 128 : (st + 1) * 128],
                            w_sbs["v"][:, dt_],
                            start=(dt_ == 0),
                            stop=(dt_ == dt_n - 1),
                        )
                    nc.vector.scalar_tensor_tensor(
                        out=vtn[:, blk * (ab // 128) + st, :, 0:DV],
                        in0=ppv.rearrange("p (h e) -> p h e", h=hc),
                        scalar=0.0,
                        in1=bvf_sb.rearrange("p (h e) -> p h e", h=hc),
                        op0=BYPASS,
                        op1=ADD,
                    )
            # Q: blk 0 projected here, blk 1 deferred into the pending queue
            do_transposes(xq, 0, xTq, 0)
            do_transposes(xq, 1, xTq, ab)
            for ck in range(ck_n):
                proj_block("q", qT, ck, xTq, 0)

        # ---------------- Phase B: attention + o_proj -----------------------
        zn = SQC // 128
        with (
            tc.tile_pool(name="ep", bufs=10) as ep_pool,
            tc.tile_pool(name="ctxu", bufs=3) as ctxu_pool,
            tc.tile_pool(name="wtsT", bufs=2) as wtsT_pool,
            tc.tile_pool(name="wnat", bufs=3) as wnat_pool,
            tc.tile_pool(name="rcp", bufs=3) as rcp_pool,
            tc.tile_pool(name="outsb", bufs=3) as outsb_pool,
            tc.tile_pool(name="sc_ps", bufs=2, space="PSUM") as sc_ps,
            tc.tile_pool(name="ctx_ps", bufs=2, space="PSUM") as ctx_ps,
            tc.tile_pool(name="aux_ps", bufs=2, space="PSUM") as aux_ps,
        ):
            # Deferred post-processing (normalization / re-transposes /
            # o_proj), emitted in ~1-2us chunks interleaved into later
            # iterations' attention loops. They draw PSUM from a dedicated
            # 2-slot aux pool so they overlap the scores/exp/PV pipeline
            # instead of stalling it.
            pending = []

            def proj_chunk(nm, outT, ck, src, c0):
                # deferred 512-column projection chunk: outT[:, ck, c0:c0+512]
                def emit():
                    pp = aux_ps.tile([128, 512], F32, name="aux")
                    for dt_ in range(dt_n):
                        nc.tensor.matmul(
                            pp,
                            w_sbs[nm][:, dt_, ck * 128 : (ck + 1) * 128],
                            src[:, dt_, c0 : c0 + 512],
                            start=(dt_ == 0), stop=(dt_ == dt_n - 1),
                        )
                    nc.vector.tensor_scalar(
                        out=outT[:, ck, c0 : c0 + 512],
                        in0=pp,
                        scalar1=bias_t[nm][:, ck : ck + 1],
                        scalar2=None,
                        op0=ADD,
                    )
                return emit

            # Deferred Q blk-1 projections (first needed by q-chunk 2;
            # drained via the t%4 pop cadence well before that)
            for ck in range(ck_n):
                pending.append(proj_chunk("q", qT, ck, xTq, 1024))
                pending.append(proj_chunk("q", qT, ck, xTq, 1536))

            def weights_chunk(q, j, m, ctxu, wnat, rc, wtsT_sb):
                def emit():
                    # padded to DV+2 so each zz slice starts 4-byte aligned
                    nat = aux_ps.tile([128, zn, DV + 2], BF16, name="aux")
                    for zz in range(zn):
                        nc.tensor.transpose(
                            nat[:, zz, 0 : DV + 1],
                            ctxu[:, m * SQC + zz * 128 : m * SQC + (zz + 1) * 128],
                            identb[0 : DV + 1, 0 : DV + 1],
                        )
                    nc.vector.reciprocal(rc[:, m], nat[:, :, DV : DV + 1])
                    for zz in range(zn):
                        nc.vector.tensor_scalar(
                            out=wnat[:, m, zz],
                            in0=nat[:, zz, 0:DV],
                            scalar1=rc[:, m, zz],
                            scalar2=None,
                            op0=MULT,
                        )
                    # normalized natural -> head-dim-major (o_proj lhsT).
                    # Both heads: transpose-mode landing at psum partition 0;
                    # head B's copy shifts partitions 0:64 -> 64:128.
                    wtp = aux_ps.tile([64, zn, 128], BF16, name="aux")
                    for zz in range(zn):
                        nc.tensor.transpose(wtp[:, zz], wnat[:, m, zz], identb)
                    nc.vector.tensor_copy(
                        wtsT_sb[m * 64 : m * 64 + 64, j, :], wtp
                    )
                    nc.sync.dma_start(out=wts_v[q, j, :, m], in_=wnat[:, m])
                return emit

            def oproj_chunk(q, zz, h2, wtsT_sb, out_sb):
                def emit():
                    op = aux_ps.tile([128, 512], F32, name="aux")
                    for dt_ in range(ck_n):
                        nc.tensor.matmul(
                            op,
                            wtsT_sb[:, dt_, zz * 128 : (zz + 1) * 128],
                            wo_sb[:, dt_, h2 * 512 : (h2 + 1) * 512],
                            start=(dt_ == 0), stop=(dt_ == ck_n - 1),
                        )
                    nc.vector.tensor_copy(out_sb[:, h2 * 512 : (h2 + 1) * 512], op)
                    if h2 == d // 512 - 1:
                        r0 = q * SQC + zz * 128
                        nc.sync.dma_start(out=out_p[r0 : r0 + 128, :], in_=out_sb)
                return emit

            for q in range(sq_n):
                q0 = q * SQC
                wtsT_sb = wtsT_pool.tile([128, ck_n, SQC], BF16, name="wtsT_sb")
                for j in range(ck_n):
                    ctxA = ctx_ps.tile([DV + 1, SQC], F32, name="ctx_t")
                    ctxB = ctx_ps.tile([DV + 1, SQC], F32, name="ctx_t")
                    for t in range(skt_n):
                        sc = sc_ps.tile([128, 2 * SQC], F32, name="sc_t")
                        for m in range(2):
                            lo, hi = m * 64, (m + 1) * 64
                            nc.tensor.matmul(
                                sc[:, m * SQC : (m + 1) * SQC],
                                kT[lo:hi, j, t * 128 : (t + 1) * 128],
                                qT[lo:hi, j, q0 : q0 + SQC],
                                start=True, stop=True,
                                tile_position=(m * 64, 0),
                            )
                        ep = ep_pool.tile([128, 2 * SQC], BF16, name="ep_t")
                        nc.scalar.activation(
                            ep, sc, EXP, bias=msk_sb[:, t : t + 1], scale=0.125
                        )
                        nc.tensor.matmul(
                            ctxA, vtn[:, t, 2 * j], ep[:, 0:SQC],
                            start=(t == 0), stop=(t == skt_n - 1),
                        )
                        nc.tensor.matmul(
                            ctxB, vtn[:, t, 2 * j + 1], ep[:, SQC : 2 * SQC],
                            start=(t == 0), stop=(t == skt_n - 1),
                        )
                        if t % 4 == 3 and pending:
                            pending.pop(0)()

                    ctxu = ctxu_pool.tile([DV + 1, 2 * SQC], BF16, name="ctxu_t")
                    nc.vector.tensor_copy(ctxu[:, 0:SQC], ctxA)
                    nc.vector.tensor_copy(ctxu[:, SQC : 2 * SQC], ctxB)
                    wnat = wnat_pool.tile([128, 2, zn, DV], BF16, name="wnat_t")
                    rc = rcp_pool.tile([128, 2, zn, 1], F32, name="rc_t")
                    for m in range(2):
                        pending.append(
                            weights_chunk(q, j, m, ctxu, wnat, rc, wtsT_sb)
                        )
                for zz in range(zn):
                    out_sb = outsb_pool.tile([128, d], F32, name="out_sb")
                    for h2 in range(d // 512):
                        pending.append(oproj_chunk(q, zz, h2, wtsT_sb, out_sb))
            while pending:
                pending.pop(0)()
    return nc


_CACHE = {}


def _get_program():
    if "nc" not in _CACHE:
        nc = bacc.Bacc("TRN2")
        build_program(nc)
        nc.compile()
        _CACHE["nc"] = nc
    return _CACHE["nc"]


def kernel(query, key, value, mask, Wq, bq, Wk, bk, Wv, bv, Wo, bo, trace=False):
    f32 = lambda a: np.ascontiguousarray(a, dtype=np.float32)
    bf16 = lambda a: np.ascontiguousarray(np.asarray(a, dtype=np.float32), dtype=NP_BF16)
    query, key, value, mask = bf16(query), bf16(key), bf16(value), f32(mask)
    Wq, Wk, Wv, Wo = bf16(Wq), bf16(Wk), bf16(Wv), bf16(Wo)
    bq, bk, bv, bo = f32(bq), f32(bk), f32(bv), f32(bo)

    in_maps = []
    for c in range(NCORES):
        b, g = c // 2, c % 2
        cols = slice(g * HDK, (g + 1) * HDK)
        bvf = np.ascontiguousarray(
            np.broadcast_to(bv[cols].astype(NP_BF16), (128, HDK))
        )
        in_maps.append({
            "xq": query[b], "xk": key[b], "xv": value[b],
            "wq": np.ascontiguousarray(Wq[:, cols]),
            "wk": np.ascontiguousarray(Wk[:, cols]),
            "wv": np.ascontiguousarray(Wv[:, cols]),
            "bq": bq[cols].reshape(HDK // 128, 128, 1),
            "bk": bk[cols].reshape(HDK // 128, 128, 1),
            "bvf": bvf,
            "wo": np.ascontiguousarray(Wo[cols, :]),
            "msk": mask[b, 0, 0].reshape(S // 128, 128, 1),
        })

    nc = _get_program()
    res = run_bass_kernel_spmd(
        nc, in_maps, core_ids=list(range(NCORES)), trace=trace
    )

    output = np.empty((B, S, D), np.float32)
    weights = np.empty((B, S, H * DV), np.float32)
    for b in range(B):
        output[b] = res.results[2 * b]["out_p"] + res.results[2 * b + 1]["out_p"] + bo
        weights[b, :, 0:HDK] = res.results[2 * b]["wts_p"].astype(np.float32)
        weights[b, :, HDK:] = res.results[2 * b + 1]["wts_p"].astype(np.float32)
    if trace:
        _CACHE["last_exec_time_ns"] = res.exec_time_ns
        _CACHE["last_res"] = res
    return output, weights


# revision 53
# speedup vs baseline: 1.1406x; 1.0270x over previous
"""Multi-head attention (B=4, S=2048, D=1024, H=16, dk=dv=64) on 8 Trainium2
NeuronCores.

Sharding: core c handles batch b = c//2 and head-group g = c%2 (8 of 16 heads).
Per core (bf16 data path, fp32 PSUM accumulation):
  - Inputs X and weights W arrive in bf16 (host-converted): DMA loads them
    directly, PE-transposes the (2048, 1024) inputs in bf16.
  - K and Q project to K^T/Q^T [512, 2048] (head-dim on partitions), biases
    folded into the PSUM->SBUF copies (bf16 out). V projects DIRECTLY to
    natural [2048, 8x(64+1)] per 128-row key tile (X^T block stationary, Wv
    moving), bias added via a host-replicated [128, 512] tile on the
    PSUM->SBUF copy; a ones column per head makes the PV matmul also produce
    softmax row-sums.
  - Per head-pair, 512-wide query chunk, key tile: scores^T = K Q^T via
    row-tiled K=64 bf16 matmuls (array rows 0:64 / 64:128) into fp32 PSUM,
    exp on the scalar engine straight out of PSUM (mask as the per-partition
    bias, 1/sqrt(dk) as the scale) writing bf16, PV accumulated in fp32
    PSUM over the 16 key tiles.
  - Context (+rowsum row) is PE-transposed (bf16) back to natural,
    normalized with the reciprocal rowsums, DMA'd out as bf16 `weights`
    (host casts to f32), then transposed once more to head-dim-major (bf16)
    as the stationary operand of the output projection. o_proj bias is
    added on the host during the cross-core reduction.
  - Pipeline balance: phase B runs the scalar engine (softmax exp, ~1.09us
    per [128,1024] tile) and the PE (scores+PV+deferred chunks) at
    equilibrium (~1.35us per key-tile iteration each). Post-processing
    (normalize/re-transpose/o_proj) and the Q blk-1 projection are emitted
    as ~1-2us deferred chunks popped every 4th key tile, which is exactly
    the PE slack left over by the exp stream -- deferring more work into
    phase B starves the scalar engine and regresses (measured).
Host: slices inputs per core (casting X, W to bf16), sums the o_proj
partials of each core pair plus bo, and concatenates the weights halves.

Measured end-to-end relative error ~9.7e-3 (tolerance 2e-2), deterministic
for the fixed-seed harness inputs.
"""
import sys

for _p in ("/opt/trn_rl_repo", "/root/.axon_site/_ro/trn_rl_repo"):
    if _p not in sys.path:
        sys.path.insert(0, _p)

import numpy as np
import ml_dtypes
import concourse.bass as bass
import concourse.bacc as bacc
import concourse.tile as tile
from concourse import mybir
from concourse.masks import make_identity
from concourse.bass_utils import run_bass_kernel_spmd

F32 = mybir.dt.float32
BF16 = mybir.dt.bfloat16
NP_BF16 = ml_dtypes.bfloat16
EXP = mybir.ActivationFunctionType.Exp
ADD = mybir.AluOpType.add
MULT = mybir.AluOpType.mult
BYPASS = mybir.AluOpType.bypass

B, S, D = 4, 2048, 1024
H, DK, DV = 16, 64, 64
NCORES = 8
HC = H // 2          # heads per core
HDK = HC * DK        # 512 head dims per core
SQC = 512            # query-chunk width


def build_program(nc: bass.Bass, s=S, d=D, hc=HC):
    hdk = hc * DK
    ck_n = hdk // 128        # dk partition-tiles (= head pairs)
    dt_n = d // 128          # D contraction tiles
    skt_n = s // 128         # key tiles
    sq_n = s // SQC          # query chunks
    ab = min(2 * SQC, s)     # phase-A S-block width
    abn = s // ab

    xq = nc.dram_tensor("xq", [s, d], BF16, kind="ExternalInput")
    xk = nc.dram_tensor("xk", [s, d], BF16, kind="ExternalInput")
    xv = nc.dram_tensor("xv", [s, d], BF16, kind="ExternalInput")
    wq = nc.dram_tensor("wq", [d, hdk], BF16, kind="ExternalInput")
    wk = nc.dram_tensor("wk", [d, hdk], BF16, kind="ExternalInput")
    wv = nc.dram_tensor("wv", [d, hdk], BF16, kind="ExternalInput")
    bq = nc.dram_tensor("bq", [ck_n, 128, 1], F32, kind="ExternalInput")
    bk = nc.dram_tensor("bk", [ck_n, 128, 1], F32, kind="ExternalInput")
    bvf = nc.dram_tensor("bvf", [128, hdk], BF16, kind="ExternalInput")
    wo = nc.dram_tensor("wo", [hdk, d], BF16, kind="ExternalInput")
    msk = nc.dram_tensor("msk", [skt_n, 128, 1], F32, kind="ExternalInput")

    out_p = nc.dram_tensor("out_p", [s, d], F32, kind="ExternalOutput")
    wts_p = nc.dram_tensor("wts_p", [s, hdk], BF16, kind="ExternalOutput")

    # weights DMA view: rows (q, z, p), cols (pair j, head m, dv)
    wts_v = wts_p.rearrange(
        "(q z p) (j m e) -> q j p m z e", z=SQC // 128, p=128, m=2, e=DV
    )

    with tile.TileContext(nc) as tc, \
            tc.tile_pool(name="consts", bufs=1) as consts, \
            tc.tile_pool(name="persist", bufs=1) as persist:
        identb = consts.tile([128, 128], BF16, name="identb")
        make_identity(nc, identb)
        msk_sb = consts.tile([128, skt_n], F32, name="msk_sb")
        nc.sync.dma_start(out=msk_sb, in_=msk.rearrange("t p one -> p (t one)"))
        bias_t = {}
        for nm, src in (("q", bq), ("k", bk)):
            bt = consts.tile([128, ck_n], F32, name=f"b{nm}_t")
            nc.sync.dma_start(out=bt, in_=src.rearrange("t p one -> p (t one)"))
            bias_t[nm] = bt
        bvf_sb = consts.tile([128, hdk], BF16, name="bvf_sb")
        nc.sync.dma_start(out=bvf_sb, in_=bvf[:])

        qT = persist.tile([128, ck_n, s], BF16, name="qT")
        kT = persist.tile([128, ck_n, s], BF16, name="kT")
        vtn = persist.tile([128, skt_n, hc, DV + 1], BF16, name="vtn")
        ones_th = consts.tile([128, skt_n * hc], BF16, name="ones_th")
        nc.gpsimd.memset(ones_th, 1.0)
        nc.vector.tensor_copy(
            vtn[:, :, :, DV : DV + 1],
            ones_th.rearrange("p (t h one) -> p t h one", t=skt_n, one=1),
        )
        wo_sb = persist.tile([128, ck_n, d], BF16, name="wo_sb")
        # QKV weights + blk-1 transposed-X staging persist into phase B: the
        # blk-1 projections are deferred into the attention pending queue so
        # the softmax stream starts earlier.
        w_sbs = {
            nm: persist.tile([128, dt_n, hdk], BF16, name=f"w{nm}_sb")
            for nm in ("k", "v", "q")
        }
        # only K's weights ahead of the X stream; V/Q/O weights DMA later
        # (interleaved into phase A, still well before their consumers)
        for t in range(dt_n):
            nc.sync.dma_start(
                out=w_sbs["k"][:, t], in_=wk[t * 128 : (t + 1) * 128, :]
            )
        xTq = persist.tile([128, dt_n, s], BF16, name="xTq")
        xTk = persist.tile([128, dt_n, s], BF16, name="xTk")

        # ---------------- Phase A: transposes + projections -----------------
        # K first, then V, then Q: attention on early query chunks only
        # needs kT + vtn + the first Q block.
        ncopy = 0
        with (
            tc.tile_pool(name="xnat", bufs=5) as xnat_pool,
            tc.tile_pool(name="xT", bufs=2) as xT_pool,
            tc.tile_pool(name="pa_ps", bufs=2, space="PSUM") as pa_ps,
            tc.tile_pool(name="tr_ps", bufs=3, space="PSUM") as tr_ps,
        ):
            def do_transposes(xz, blk, dst_sb, dst_c0):
                # transpose rows [blk*ab, (blk+1)*ab) of xz into dst_sb
                # (layout [128, dt_n, cols]) starting at column dst_c0
                nonlocal ncopy
                for st in range(ab // 128):
                    x_sub = xnat_pool.tile([128, d], BF16, name="x_sub")
                    s0 = blk * ab + st * 128
                    nc.sync.dma_start(out=x_sub, in_=xz[s0 : s0 + 128, :])
                    for tg in range(dt_n // 4):
                        tp = tr_ps.tile([128, 4, 128], BF16, name="tr_tp")
                        for j in range(4):
                            nc.tensor.transpose(
                                tp[:, j],
                                x_sub[:, (tg * 4 + j) * 128 : (tg * 4 + j + 1) * 128],
                                identb,
                            )
                        c0 = dst_c0 + st * 128
                        dst = dst_sb[:, tg * 4 : tg * 4 + 4, c0 : c0 + 128]
                        if ncopy % 2 == 0:
                            nc.vector.tensor_copy(dst, tp)
                        else:
                            nc.scalar.copy(dst, tp)
                        ncopy += 1

            def proj_block(nm, outT, ck, src, c0):
                # project 1024 columns [c0, c0+ab) of src into outT[:, ck]
                pp = pa_ps.tile([128, ab], F32, name="proj_pp")
                for dt_ in range(dt_n):
                    lhsT = w_sbs[nm][:, dt_, ck * 128 : (ck + 1) * 128]
                    for h2 in range(ab // 512):
                        nc.tensor.matmul(
                            pp[:, h2 * 512 : (h2 + 1) * 512],
                            lhsT,
                            src[:, dt_, c0 + h2 * 512 : c0 + (h2 + 1) * 512],
                            start=(dt_ == 0),
                            stop=(dt_ == dt_n - 1),
                        )
                nc.vector.tensor_scalar(
                    out=outT[:, ck, c0 : c0 + ab],
                    in0=pp,
                    scalar1=bias_t[nm][:, ck : ck + 1],
                    scalar2=None,
                    op0=ADD,
                )

            # K: fully projected here
            do_transposes(xk, 0, xTk, 0)
            for t in range(dt_n):
                nc.sync.dma_start(
                    out=w_sbs["v"][:, t], in_=wv[t * 128 : (t + 1) * 128, :]
                )
            do_transposes(xk, 1, xTk, ab)
            for t in range(dt_n):
                nc.sync.dma_start(
                    out=w_sbs["q"][:, t], in_=wq[t * 128 : (t + 1) * 128, :]
                )
            for t in range(ck_n):
                nc.sync.dma_start(
                    out=wo_sb[:, t], in_=wo[t * 128 : (t + 1) * 128, :]
                )
            for blk in range(abn):
                for ck in range(ck_n):
                    proj_block("k", kT, ck, xTk, blk * ab)
            # V: fully projected here (every attention iteration reads all
            # key tiles, so V cannot be deferred without starving softmax).
            for blk in range(abn):
                xTv = xT_pool.tile([128, dt_n, ab], BF16, name="xT_sb")
                do_transposes(xv, blk, xTv, 0)
                for st in range(ab // 128):
                    ppv = pa_ps.tile([128, ab], F32, name="proj_pp")[:, 0:hdk]
                    for dt_ in range(dt_n):
                        nc.tensor.matmul(
                            ppv,
                            xTv[:, dt_, st * 128 : (st + 1) * 128],
                            w_sbs["v"][:, dt_],
                            start=(dt_ == 0),
                            stop=(dt_ == dt_n - 1),
                        )
                    nc.vector.scalar_tensor_tensor(
                        out=vtn[:, blk * (ab // 128) + st, :, 0:DV],
                        in0=ppv.rearrange("p (h e) -> p h e", h=hc),
                        scalar=0.0,
                        in1=bvf_sb.rearrange("p (h e) -> p h e", h=hc),
                        op0=BYPASS,
                        op1=ADD,
                    )
            # Q: blk 0 projected here, blk 1 deferred into the pending queue
            do_transposes(xq, 0, xTq, 0)
            do_transposes(xq, 1, xTq, ab)
            for ck in range(ck_n):
                proj_block("q", qT, ck, xTq, 0)

        # ---------------- Phase B: attention + o_proj -----------------------
        zn = SQC // 128
        with (
            tc.tile_pool(name="ep", bufs=10) as ep_pool,
            tc.tile_pool(name="ctxu", bufs=3) as ctxu_pool,
            tc.tile_pool(name="wtsT", bufs=2) as wtsT_pool,
            tc.tile_pool(name="wnat", bufs=3) as wnat_pool,
            tc.tile_pool(name="rcp", bufs=3) as rcp_pool,
            tc.tile_pool(name="outsb", bufs=3) as outsb_pool,
            tc.tile_pool(name="sc_ps", bufs=2, space="PSUM") as sc_ps,
            tc.tile_pool(name="ctx_ps", bufs=2, space="PSUM") as ctx_ps,
            tc.tile_pool(name="aux_ps", bufs=2, space="PSUM") as aux_ps,
        ):
            # Deferred post-processing (normalization / re-transposes /
            # o_proj), emitted in ~1-2us chunks interleaved into later
            # iterations' attention loops. They draw PSUM from a dedicated
            # 2-slot aux pool so they overlap the scores/exp/PV pipeline
            # instead of stalling it.
            pending = []

            def proj_chunk(nm, outT, ck, src, c0):
                # deferred 512-column projection chunk: outT[:, ck, c0:c0+512]
                def emit():
                    pp = aux_ps.tile([128, 512], F32, name="aux")
                    for dt_ in range(dt_n):
                        nc.tensor.matmul(
                            pp,
                            w_sbs[nm][:, dt_, ck * 128 : (ck + 1) * 128],
                            src[:, dt_, c0 : c0 + 512],
                            start=(dt_ == 0), stop=(dt_ == dt_n - 1),
                        )
                    nc.vector.tensor_scalar(
                        out=outT[:, ck, c0 : c0 + 512],
                        in0=pp,
                        scalar1=bias_t[nm][:, ck : ck + 1],
                        scalar2=None,
                        op0=ADD,
                    )
                return emit

            # Deferred Q blk-1 projections (first needed by q-chunk 2;
            # drained via the t%4 pop cadence well before that)
            for ck in range(ck_n):
                pending.append(proj_chunk("q", qT, ck, xTq, 1024))
                pending.append(proj_chunk("q", qT, ck, xTq, 1536))

            def weights_chunk(q, j, m, ctxu, wnat, rc, wtsT_sb):
                def emit():
                    # padded to DV+2 so each zz slice starts 4-byte aligned
                    nat = aux_ps.tile([128, zn, DV + 2], BF16, name="aux")
                    for zz in range(zn):
                        nc.tensor.transpose(
                            nat[:, zz, 0 : DV + 1],
                            ctxu[:, m * SQC + zz * 128 : m * SQC + (zz + 1) * 128],
                            identb[0 : DV + 1, 0 : DV + 1],
                        )
                    nc.vector.reciprocal(rc[:, m], nat[:, :, DV : DV + 1])
                    for zz in range(zn):
                        nc.vector.tensor_scalar(
                            out=wnat[:, m, zz],
                            in0=nat[:, zz, 0:DV],
                            scalar1=rc[:, m, zz],
                            scalar2=None,
                            op0=MULT,
                        )
                    # normalized natural -> head-dim-major (o_proj lhsT).
                    # Both heads: transpose-mode landing at psum partition 0;
                    # head B's copy shifts partitions 0:64 -> 64:128.
                    wtp = aux_ps.tile([64, zn, 128], BF16, name="aux")
                    for zz in range(zn):
                        nc.tensor.transpose(wtp[:, zz], wnat[:, m, zz], identb)
                    nc.vector.tensor_copy(
                        wtsT_sb[m * 64 : m * 64 + 64, j, :], wtp
                    )
                    nc.sync.dma_start(out=wts_v[q, j, :, m], in_=wnat[:, m])
                return emit

            def oproj_chunk(q, zz, h2, wtsT_sb, out_sb):
                def emit():
                    op = aux_ps.tile([128, 512], F32, name="aux")
                    for dt_ in range(ck_n):
                        nc.tensor.matmul(
                            op,
                            wtsT_sb[:, dt_, zz * 128 : (zz + 1) * 128],
                            wo_sb[:, dt_, h2 * 512 : (h2 + 1) * 512],
                            start=(dt_ == 0), stop=(dt_ == ck_n - 1),
                        )
                    nc.vector.tensor_copy(out_sb[:, h2 * 512 : (h2 + 1) * 512], op)
                    if h2 == d // 512 - 1:
                        r0 = q * SQC + zz * 128
                        nc.sync.dma_start(out=out_p[r0 : r0 + 128, :], in_=out_sb)
                return emit

            for q in range(sq_n):
                q0 = q * SQC
                wtsT_sb = wtsT_pool.tile([128, ck_n, SQC], BF16, name="wtsT_sb")
                for j in range(ck_n):
                    ctxA = ctx_ps.tile([DV + 1, SQC], F32, name="ctx_t")
                    ctxB = ctx_ps.tile([DV + 1, SQC], F32, name="ctx_t")
                    for t in range(skt_n):
                        sc = sc_ps.tile([128, 2 * SQC], F32, name="sc_t")
                        for m in range(2):
                            lo, hi = m * 64, (m + 1) * 64
                            nc.tensor.matmul(
                                sc[:, m * SQC : (m + 1) * SQC],
                                kT[lo:hi, j, t * 128 : (t + 1) * 128],
                                qT[lo:hi, j, q0 : q0 + SQC],
                                start=True, stop=True,
                                tile_position=(m * 64, 0),
                            )
                        ep = ep_pool.tile([128, 2 * SQC], BF16, name="ep_t")
                        nc.scalar.activation(
                            ep, sc, EXP, bias=msk_sb[:, t : t + 1], scale=0.125
                        )
                        nc.tensor.matmul(
                            ctxA, vtn[:, t, 2 * j], ep[:, 0:SQC],
                            start=(t == 0), stop=(t == skt_n - 1),
                        )
                        nc.tensor.matmul(
                            ctxB, vtn[:, t, 2 * j + 1], ep[:, SQC : 2 * SQC],
                            start=(t == 0), stop=(t == skt_n - 1),
                        )
                        if t % 4 == 3 and pending:
                            pending.pop(0)()

                    ctxu = ctxu_pool.tile([DV + 1, 2 * SQC], BF16, name="ctxu_t")
                    nc.vector.tensor_copy(ctxu[:, 0:SQC], ctxA)
                    nc.vector.tensor_copy(ctxu[:, SQC : 2 * SQC], ctxB)
                    wnat = wnat_pool.tile([128, 2, zn, DV], BF16, name="wnat_t")
                    rc = rcp_pool.tile([128, 2, zn, 1], F32, name="rc_t")
                    for m in range(2):
                        pending.append(
                            weights_chunk(q, j, m, ctxu, wnat, rc, wtsT_sb)
                        )
                for zz in range(zn):
                    out_sb = outsb_pool.tile([128, d], F32, name="out_sb")
                    for h2 in range(d // 512):
                        pending.append(oproj_chunk(q, zz, h2, wtsT_sb, out_sb))
            while pending:
                pending.pop(0)()
    return nc


_CACHE = {}


def _get_program():
    if "nc" not in _CACHE:
        nc = bacc.Bacc("TRN2")
        build_program(nc)
        nc.compile()
        _CACHE["nc"] = nc
    return _CACHE["nc"]


def kernel(query, key, value, mask, Wq, bq, Wk, bk, Wv, bv, Wo, bo, trace=False):
    f32 = lambda a: np.ascontiguousarray(a, dtype=np.float32)
    bf16 = lambda a: np.ascontiguousarray(np.asarray(a, dtype=np.float32), dtype=NP_BF16)
    query, key, value, mask = bf16(query), bf16(key), bf16(value), f32(mask)
    Wq, Wk, Wv, Wo = bf16(Wq), bf16(Wk), bf16(Wv), bf16(Wo)
    bq, bk, bv, bo = f32(bq), f32(bk), f32(bv), f32(bo)

    in_maps = []
    for c in range(NCORES):
        b, g = c // 2, c % 2
        cols = slice(g * HDK, (g + 1) * HDK)
        bvf = np.ascontiguousarray(
            np.broadcast_to(bv[cols].astype(NP_BF16), (128, HDK))
        )
        in_maps.append({
            "xq": query[b], "xk": key[b], "xv": value[b],
            "wq": np.ascontiguousarray(Wq[:, cols]),
            "wk": np.ascontiguousarray(Wk[:, cols]),
            "wv": np.ascontiguousarray(Wv[:, cols]),
            "bq": bq[cols].reshape(HDK // 128, 128, 1),
            "bk": bk[cols].reshape(HDK // 128, 128, 1),
            "bvf": bvf,
            "wo": np.ascontiguousarray(Wo[cols, :]),
            "msk": mask[b, 0, 0].reshape(S // 128, 128, 1),
        })

    nc = _get_program()
    res = run_bass_kernel_spmd(
        nc, in_maps, core_ids=list(range(NCORES)), trace=trace
    )

    output = np.empty((B, S, D), np.float32)
    weights = np.empty((B, S, H * DV), np.float32)
    for b in range(B):
        output[b] = res.results[2 * b]["out_p"] + res.results[2 * b + 1]["out_p"] + bo
        weights[b, :, 0:HDK] = res.results[2 * b]["wts_p"].astype(np.float32)
        weights[b, :, HDK:] = res.results[2 * b + 1]["wts_p"].astype(np.float32)
    if trace:
        _CACHE["last_exec_time_ns"] = res.exec_time_ns
        _CACHE["last_res"] = res
    return output, weights


# revision 58
# speedup vs baseline: 1.1707x; 1.0264x over previous
"""Multi-head attention (B=4, S=2048, D=1024, H=16, dk=dv=64) on 8 Trainium2
NeuronCores.

Sharding: core c handles batch b = c//2 and head-group g = c%2 (8 of 16 heads).
Per core (bf16 data path, fp32 PSUM accumulation):
  - Inputs X and weights W arrive in bf16 (host-converted): DMA loads them
    directly, PE-transposes the (2048, 1024) inputs in bf16.
  - K and Q project to K^T/Q^T [512, 2048] (head-dim on partitions), biases
    folded into the PSUM->SBUF copies (bf16 out). V projects DIRECTLY to
    natural [2048, 8x(64+1)] per 128-row key tile (X^T block stationary, Wv
    moving), bias added via a host-replicated [128, 512] tile on the
    PSUM->SBUF copy; a ones column per head makes the PV matmul also produce
    softmax row-sums.
  - Per head-pair, 512-wide query chunk, key tile: scores^T = K Q^T via
    row-tiled K=64 bf16 matmuls (array rows 0:64 / 64:128) into fp32 PSUM,
    exp on the scalar engine straight out of PSUM (mask as the per-partition
    bias, 1/sqrt(dk) as the scale) writing bf16, PV accumulated in fp32
    PSUM over the 16 key tiles.
  - Context (+rowsum row) is PE-transposed (bf16) back to natural,
    normalized with the reciprocal rowsums, DMA'd out as bf16 `weights`
    (host casts to f32), then transposed once more to head-dim-major (bf16)
    as the stationary operand of the output projection. o_proj bias is
    added on the host during the cross-core reduction.
  - Pipeline balance: phase B runs the scalar engine (softmax exp, ~1.09us
    per [128,1024] tile) and the PE (scores+PV+deferred chunks) at
    equilibrium (~1.35us per key-tile iteration each). Post-processing
    (normalize/re-transpose/o_proj) and the Q blk-1 projection are emitted
    as ~1-2us deferred chunks popped every 4th key tile, which is exactly
    the PE slack left over by the exp stream -- deferring more work into
    phase B starves the scalar engine and regresses (measured).
Host: slices inputs per core (casting X, W to bf16), sums the o_proj
partials of each core pair plus bo, and concatenates the weights halves.

Measured end-to-end relative error ~9.7e-3 (tolerance 2e-2), deterministic
for the fixed-seed harness inputs.
"""
import sys

for _p in ("/opt/trn_rl_repo", "/root/.axon_site/_ro/trn_rl_repo"):
    if _p not in sys.path:
        sys.path.insert(0, _p)

import numpy as np
import ml_dtypes
import concourse.bass as bass
import concourse.bacc as bacc
import concourse.tile as tile
from concourse import mybir
from concourse.masks import make_identity
from concourse.bass_utils import run_bass_kernel_spmd

F32 = mybir.dt.float32
BF16 = mybir.dt.bfloat16
NP_BF16 = ml_dtypes.bfloat16
EXP = mybir.ActivationFunctionType.Exp
ADD = mybir.AluOpType.add
MULT = mybir.AluOpType.mult
BYPASS = mybir.AluOpType.bypass

B, S, D = 4, 2048, 1024
H, DK, DV = 16, 64, 64
NCORES = 8
HC = H // 2          # heads per core
HDK = HC * DK        # 512 head dims per core
SQC = 512            # query-chunk width


def build_program(nc: bass.Bass, s=S, d=D, hc=HC):
    hdk = hc * DK
    ck_n = hdk // 128        # dk partition-tiles (= head pairs)
    dt_n = d // 128          # D contraction tiles
    skt_n = s // 128         # key tiles
    sq_n = s // SQC          # query chunks
    ab = min(2 * SQC, s)     # phase-A S-block width
    abn = s // ab

    xq = nc.dram_tensor("xq", [s, d], BF16, kind="ExternalInput")
    xk = nc.dram_tensor("xk", [s, d], BF16, kind="ExternalInput")
    xv = nc.dram_tensor("xv", [s, d], BF16, kind="ExternalInput")
    wq = nc.dram_tensor("wq", [d, hdk], BF16, kind="ExternalInput")
    wk = nc.dram_tensor("wk", [d, hdk], BF16, kind="ExternalInput")
    wv = nc.dram_tensor("wv", [d, hdk], BF16, kind="ExternalInput")
    bq = nc.dram_tensor("bq", [ck_n, 128, 1], F32, kind="ExternalInput")
    bk = nc.dram_tensor("bk", [ck_n, 128, 1], F32, kind="ExternalInput")
    bvf = nc.dram_tensor("bvf", [128, hdk], BF16, kind="ExternalInput")
    wo = nc.dram_tensor("wo", [hdk, d], BF16, kind="ExternalInput")
    msk = nc.dram_tensor("msk", [skt_n, 128, 1], F32, kind="ExternalInput")

    out_p = nc.dram_tensor("out_p", [s, d], F32, kind="ExternalOutput")
    wts_p = nc.dram_tensor("wts_p", [s, hdk], BF16, kind="ExternalOutput")

    # weights DMA view: rows (q, z, p), cols (pair j, head m, dv)
    wts_v = wts_p.rearrange(
        "(q z p) (j m e) -> q j p m z e", z=SQC // 128, p=128, m=2, e=DV
    )

    with tile.TileContext(nc) as tc, \
            tc.tile_pool(name="consts", bufs=1) as consts, \
            tc.tile_pool(name="persist", bufs=1) as persist:
        identb = consts.tile([128, 128], BF16, name="identb")
        make_identity(nc, identb)
        msk_sb = consts.tile([128, skt_n], F32, name="msk_sb")
        nc.sync.dma_start(out=msk_sb, in_=msk.rearrange("t p one -> p (t one)"))
        bias_t = {}
        for nm, src in (("q", bq), ("k", bk)):
            bt = consts.tile([128, ck_n], F32, name=f"b{nm}_t")
            nc.sync.dma_start(out=bt, in_=src.rearrange("t p one -> p (t one)"))
            bias_t[nm] = bt
        bvf_sb = consts.tile([128, hdk], BF16, name="bvf_sb")
        nc.sync.dma_start(out=bvf_sb, in_=bvf[:])

        qT = persist.tile([128, ck_n, s], BF16, name="qT")
        kT = persist.tile([128, ck_n, s], BF16, name="kT")
        vtn = persist.tile([128, skt_n, hc, DV + 1], BF16, name="vtn")
        ones_th = consts.tile([128, skt_n * hc], BF16, name="ones_th")
        nc.gpsimd.memset(ones_th, 1.0)
        nc.vector.tensor_copy(
            vtn[:, :, :, DV : DV + 1],
            ones_th.rearrange("p (t h one) -> p t h one", t=skt_n, one=1),
        )
        wo_sb = persist.tile([128, ck_n, d], BF16, name="wo_sb")
        # QKV weights + blk-1 transposed-X staging persist into phase B: the
        # blk-1 projections are deferred into the attention pending queue so
        # the softmax stream starts earlier.
        w_sbs = {
            nm: persist.tile([128, dt_n, hdk], BF16, name=f"w{nm}_sb")
            for nm in ("k", "v", "q")
        }
        # only K's weights ahead of the X stream; V/Q/O weights DMA later
        # (interleaved into phase A, still well before their consumers)
        for t in range(dt_n):
            nc.sync.dma_start(
                out=w_sbs["k"][:, t], in_=wk[t * 128 : (t + 1) * 128, :]
            )
        xTq = persist.tile([128, dt_n, s], BF16, name="xTq")
        xTk = persist.tile([128, dt_n, s], BF16, name="xTk")

        # ---------------- Phase A: transposes + projections -----------------
        # K first, then V, then Q: attention on early query chunks only
        # needs kT + vtn + the first Q block.
        ncopy = 0
        with (
            tc.tile_pool(name="xnat", bufs=6) as xnat_pool,
            tc.tile_pool(name="xT", bufs=2) as xT_pool,
            tc.tile_pool(name="pa_ps", bufs=2, space="PSUM") as pa_ps,
            tc.tile_pool(name="tr_ps", bufs=4, space="PSUM") as tr_ps,
        ):
            def do_transposes(xz, blk, dst_sb, dst_c0, vector_only=False):
                # transpose rows [blk*ab, (blk+1)*ab) of xz into dst_sb
                # (layout [128, dt_n, cols]) starting at column dst_c0.
                # vector_only keeps the scalar queue clean ahead of the
                # softmax stream (used for the last input).
                nonlocal ncopy
                for st in range(ab // 128):
                    x_sub = xnat_pool.tile([128, d], BF16, name="x_sub")
                    s0 = blk * ab + st * 128
                    nc.sync.dma_start(out=x_sub, in_=xz[s0 : s0 + 128, :])
                    for tg in range(dt_n // 4):
                        tp = tr_ps.tile([128, 4, 128], BF16, name="tr_tp")
                        for j in range(4):
                            nc.tensor.transpose(
                                tp[:, j],
                                x_sub[:, (tg * 4 + j) * 128 : (tg * 4 + j + 1) * 128],
                                identb,
                            )
                        c0 = dst_c0 + st * 128
                        dst = dst_sb[:, tg * 4 : tg * 4 + 4, c0 : c0 + 128]
                        if vector_only or ncopy % 2 == 0:
                            nc.vector.tensor_copy(dst, tp)
                        else:
                            nc.scalar.copy(dst, tp)
                        ncopy += 1

            def proj_block(nm, outT, ck, src, c0):
                # project 1024 columns [c0, c0+ab) of src into outT[:, ck]
                pp = pa_ps.tile([128, ab], F32, name="proj_pp")
                for dt_ in range(dt_n):
                    lhsT = w_sbs[nm][:, dt_, ck * 128 : (ck + 1) * 128]
                    for h2 in range(ab // 512):
                        nc.tensor.matmul(
                            pp[:, h2 * 512 : (h2 + 1) * 512],
                            lhsT,
                            src[:, dt_, c0 + h2 * 512 : c0 + (h2 + 1) * 512],
                            start=(dt_ == 0),
                            stop=(dt_ == dt_n - 1),
                        )
                nc.vector.tensor_scalar(
                    out=outT[:, ck, c0 : c0 + ab],
                    in0=pp,
                    scalar1=bias_t[nm][:, ck : ck + 1],
                    scalar2=None,
                    op0=ADD,
                )

            # K: fully projected here
            do_transposes(xk, 0, xTk, 0)
            for t in range(dt_n):
                nc.sync.dma_start(
                    out=w_sbs["v"][:, t], in_=wv[t * 128 : (t + 1) * 128, :]
                )
            do_transposes(xk, 1, xTk, ab)
            for t in range(dt_n):
                nc.sync.dma_start(
                    out=w_sbs["q"][:, t], in_=wq[t * 128 : (t + 1) * 128, :]
                )
            for t in range(ck_n):
                nc.sync.dma_start(
                    out=wo_sb[:, t], in_=wo[t * 128 : (t + 1) * 128, :]
                )
            for blk in range(abn):
                for ck in range(ck_n):
                    proj_block("k", kT, ck, xTk, blk * ab)
            # V: fully projected here (every attention iteration reads all
            # key tiles, so V cannot be deferred without starving softmax).
            for blk in range(abn):
                xTv = xT_pool.tile([128, dt_n, ab], BF16, name="xT_sb")
                do_transposes(xv, blk, xTv, 0)
                for st in range(ab // 128):
                    ppv = pa_ps.tile([128, ab], F32, name="proj_pp")[:, 0:hdk]
                    for dt_ in range(dt_n):
                        nc.tensor.matmul(
                            ppv,
                            xTv[:, dt_, st * 128 : (st + 1) * 128],
                            w_sbs["v"][:, dt_],
                            start=(dt_ == 0),
                            stop=(dt_ == dt_n - 1),
                        )
                    nc.vector.scalar_tensor_tensor(
                        out=vtn[:, blk * (ab // 128) + st, :, 0:DV],
                        in0=ppv.rearrange("p (h e) -> p h e", h=hc),
                        scalar=0.0,
                        in1=bvf_sb.rearrange("p (h e) -> p h e", h=hc),
                        op0=BYPASS,
                        op1=ADD,
                    )
            # Q: blk 0 projected here, blk 1 deferred into the pending queue
            do_transposes(xq, 0, xTq, 0, vector_only=True)
            do_transposes(xq, 1, xTq, ab, vector_only=True)
            for ck in range(ck_n):
                proj_block("q", qT, ck, xTq, 0)

        # ---------------- Phase B: attention + o_proj -----------------------
        zn = SQC // 128
        with (
            tc.tile_pool(name="ep", bufs=10) as ep_pool,
            tc.tile_pool(name="ctxu", bufs=3) as ctxu_pool,
            tc.tile_pool(name="wtsT", bufs=2) as wtsT_pool,
            tc.tile_pool(name="wnat", bufs=3) as wnat_pool,
            tc.tile_pool(name="rcp", bufs=3) as rcp_pool,
            tc.tile_pool(name="outsb", bufs=3) as outsb_pool,
            tc.tile_pool(name="sc_ps", bufs=2, space="PSUM") as sc_ps,
            tc.tile_pool(name="ctx_ps", bufs=2, space="PSUM") as ctx_ps,
            tc.tile_pool(name="aux_ps", bufs=2, space="PSUM") as aux_ps,
        ):
            # Deferred post-processing (normalization / re-transposes /
            # o_proj), emitted in ~1-2us chunks interleaved into later
            # iterations' attention loops. They draw PSUM from a dedicated
            # 2-slot aux pool so they overlap the scores/exp/PV pipeline
            # instead of stalling it.
            pending = []

            def proj_chunk(nm, outT, ck, src, c0):
                # deferred 512-column projection chunk: outT[:, ck, c0:c0+512]
                def emit():
                    pp = aux_ps.tile([128, 512], F32, name="aux")
                    for dt_ in range(dt_n):
                        nc.tensor.matmul(
                            pp,
                            w_sbs[nm][:, dt_, ck * 128 : (ck + 1) * 128],
                            src[:, dt_, c0 : c0 + 512],
                            start=(dt_ == 0), stop=(dt_ == dt_n - 1),
                        )
                    nc.vector.tensor_scalar(
                        out=outT[:, ck, c0 : c0 + 512],
                        in0=pp,
                        scalar1=bias_t[nm][:, ck : ck + 1],
                        scalar2=None,
                        op0=ADD,
                    )
                return emit

            # Deferred Q blk-1 projections (first needed by q-chunk 2;
            # drained via the t%4 pop cadence well before that)
            for ck in range(ck_n):
                pending.append(proj_chunk("q", qT, ck, xTq, 1024))
                pending.append(proj_chunk("q", qT, ck, xTq, 1536))

            def weights_chunk(q, j, m, ctxu, wnat, rc, wtsT_sb):
                def emit():
                    # padded to DV+2 so each zz slice starts 4-byte aligned
                    nat = aux_ps.tile([128, zn, DV + 2], BF16, name="aux")
                    for zz in range(zn):
                        nc.tensor.transpose(
                            nat[:, zz, 0 : DV + 1],
                            ctxu[:, m * SQC + zz * 128 : m * SQC + (zz + 1) * 128],
                            identb[0 : DV + 1, 0 : DV + 1],
                        )
                    nc.vector.reciprocal(rc[:, m], nat[:, :, DV : DV + 1])
                    for zz in range(zn):
                        nc.vector.tensor_scalar(
                            out=wnat[:, m, zz],
                            in0=nat[:, zz, 0:DV],
                            scalar1=rc[:, m, zz],
                            scalar2=None,
                            op0=MULT,
                        )
                    # normalized natural -> head-dim-major (o_proj lhsT).
                    # Both heads: transpose-mode landing at psum partition 0;
                    # head B's copy shifts partitions 0:64 -> 64:128.
                    wtp = aux_ps.tile([64, zn, 128], BF16, name="aux")
                    for zz in range(zn):
                        nc.tensor.transpose(wtp[:, zz], wnat[:, m, zz], identb)
                    nc.vector.tensor_copy(
                        wtsT_sb[m * 64 : m * 64 + 64, j, :], wtp
                    )
                    nc.sync.dma_start(out=wts_v[q, j, :, m], in_=wnat[:, m])
                return emit

            def oproj_chunk(q, zz, h2, wtsT_sb, out_sb):
                def emit():
                    op = aux_ps.tile([128, 512], F32, name="aux")
                    for dt_ in range(ck_n):
                        nc.tensor.matmul(
                            op,
                            wtsT_sb[:, dt_, zz * 128 : (zz + 1) * 128],
                            wo_sb[:, dt_, h2 * 512 : (h2 + 1) * 512],
                            start=(dt_ == 0), stop=(dt_ == ck_n - 1),
                        )
                    nc.vector.tensor_copy(out_sb[:, h2 * 512 : (h2 + 1) * 512], op)
                    if h2 == d // 512 - 1:
                        r0 = q * SQC + zz * 128
                        nc.sync.dma_start(out=out_p[r0 : r0 + 128, :], in_=out_sb)
                return emit

            for q in range(sq_n):
                q0 = q * SQC
                wtsT_sb = wtsT_pool.tile([128, ck_n, SQC], BF16, name="wtsT_sb")
                for j in range(ck_n):
                    ctxA = ctx_ps.tile([DV + 1, SQC], F32, name="ctx_t")
                    ctxB = ctx_ps.tile([DV + 1, SQC], F32, name="ctx_t")
                    for t in range(skt_n):
                        sc = sc_ps.tile([128, 2 * SQC], F32, name="sc_t")
                        for m in range(2):
                            lo, hi = m * 64, (m + 1) * 64
                            nc.tensor.matmul(
                                sc[:, m * SQC : (m + 1) * SQC],
                                kT[lo:hi, j, t * 128 : (t + 1) * 128],
                                qT[lo:hi, j, q0 : q0 + SQC],
                                start=True, stop=True,
                                tile_position=(m * 64, 0),
                            )
                        ep = ep_pool.tile([128, 2 * SQC], BF16, name="ep_t")
                        nc.scalar.activation(
                            ep, sc, EXP, bias=msk_sb[:, t : t + 1], scale=0.125
                        )
                        nc.tensor.matmul(
                            ctxA, vtn[:, t, 2 * j], ep[:, 0:SQC],
                            start=(t == 0), stop=(t == skt_n - 1),
                        )
                        nc.tensor.matmul(
                            ctxB, vtn[:, t, 2 * j + 1], ep[:, SQC : 2 * SQC],
                            start=(t == 0), stop=(t == skt_n - 1),
                        )
                        if t % 4 == 3 and pending:
                            pending.pop(0)()

                    ctxu = ctxu_pool.tile([DV + 1, 2 * SQC], BF16, name="ctxu_t")
                    nc.vector.tensor_copy(ctxu[:, 0:SQC], ctxA)
                    nc.vector.tensor_copy(ctxu[:, SQC : 2 * SQC], ctxB)
                    wnat = wnat_pool.tile([128, 2, zn, DV], BF16, name="wnat_t")
                    rc = rcp_pool.tile([128, 2, zn, 1], F32, name="rc_t")
                    for m in range(2):
                        pending.append(
                            weights_chunk(q, j, m, ctxu, wnat, rc, wtsT_sb)
                        )
                for zz in range(zn):
                    out_sb = outsb_pool.tile([128, d], F32, name="out_sb")
                    for h2 in range(d // 512):
                        pending.append(oproj_chunk(q, zz, h2, wtsT_sb, out_sb))
            while pending:
                pending.pop(0)()
    return nc


_CACHE = {}


def _get_program():
    if "nc" not in _CACHE:
        nc = bacc.Bacc("TRN2")
        build_program(nc)
        nc.compile()
        _CACHE["nc"] = nc
    return _CACHE["nc"]


def kernel(query, key, value, mask, Wq, bq, Wk, bk, Wv, bv, Wo, bo, trace=False):
    f32 = lambda a: np.ascontiguousarray(a, dtype=np.float32)
    bf16 = lambda a: np.ascontiguousarray(np.asarray(a, dtype=np.float32), dtype=NP_BF16)
    query, key, value, mask = bf16(query), bf16(key), bf16(value), f32(mask)
    Wq, Wk, Wv, Wo = bf16(Wq), bf16(Wk), bf16(Wv), bf16(Wo)
    bq, bk, bv, bo = f32(bq), f32(bk), f32(bv), f32(bo)

    in_maps = []
    for c in range(NCORES):
        b, g = c // 2, c % 2
        cols = slice(g * HDK, (g + 1) * HDK)
        bvf = np.ascontiguousarray(
            np.broadcast_to(bv[cols].astype(NP_BF16), (128, HDK))
        )
        in_maps.append({
            "xq": query[b], "xk": key[b], "xv": value[b],
            "wq": np.ascontiguousarray(Wq[:, cols]),
            "wk": np.ascontiguousarray(Wk[:, cols]),
            "wv": np.ascontiguousarray(Wv[:, cols]),
            "bq": bq[cols].reshape(HDK // 128, 128, 1),
            "bk": bk[cols].reshape(HDK // 128, 128, 1),
            "bvf": bvf,
            "wo": np.ascontiguousarray(Wo[cols, :]),
            "msk": mask[b, 0, 0].reshape(S // 128, 128, 1),
        })

    nc = _get_program()
    res = run_bass_kernel_spmd(
        nc, in_maps, core_ids=list(range(NCORES)), trace=trace
    )

    output = np.empty((B, S, D), np.float32)
    weights = np.empty((B, S, H * DV), np.float32)
    for b in range(B):
        output[b] = res.results[2 * b]["out_p"] + res.results[2 * b + 1]["out_p"] + bo
        weights[b, :, 0:HDK] = res.results[2 * b]["wts_p"].astype(np.float32)
        weights[b, :, HDK:] = res.results[2 * b + 1]["wts_p"].astype(np.float32)
    if trace:
        _CACHE["last_exec_time_ns"] = res.exec_time_ns
        _CACHE["last_res"] = res
    return output, weights
